# revision 8
# baseline (speedup 1.0000x reference)
"""DMGCN message-passing GNN on 8 Trainium2 NeuronCores (Bass/Tile).

Sharding: edges sorted by dst; core c owns nodes [c*12500,(c+1)*12500) and the
edges targeting them. Per layer: node-MLP on own node shard -> AllGather hn
table -> edge phase (indirect-DMA gather hn[src], edge MLP, message matmul,
one-hot scatter matmuls accumulating in PSUM) -> h update. Readout on device,
graph segment-sum on host (unshard step).
"""
import os
import sys

for _p in ("/opt/trn_rl_repo", "/root/.axon_site/_ro/trn_rl_repo"):
    if os.path.isdir(_p) and _p not in sys.path:
        sys.path.insert(0, _p)

import numpy as np
import concourse.bass as bass
import concourse.mybir as mybir
import concourse.tile as tile
from concourse.bass_utils import run_bass_kernel_spmd
from concourse.masks import make_identity

# problem constants (hardcoded per spec)
N, E, G = 100000, 400000, 2000
D = 128
NC = 300           # RBF centers
CUT_LO, CUT_HI = 0.0, 30.0
N_CONV = 3
NCORES = 8
P = 128
N_SH = N // NCORES            # 12500 nodes per core
NT = (N_SH + P - 1) // P      # 98 node tiles per core
N_PAD = NT * P                # 12544 padded hn-table rows per core
DE = 428

F32 = mybir.dt.float32
I32 = mybir.dt.int32
AF = mybir.ActivationFunctionType
ALU = mybir.AluOpType

PAD_OFF = 200.0               # dst_off sentinel for padded edges

# K-chunking of the 428-dim edge feature axis: emb 0:128 | rbf 128:428
KCH = [(0, 128), (128, 256), (256, 384), (384, 428)]     # z1 M-chunks / z2 K-chunks
VCH = [(0, 128), (128, 256), (256, 300)]                 # rbf center chunks


def split_waits(nc):
    """Walrus allows only 1 sync wait per instruction; hoist extras onto
    preceding NoOps on the same engine."""
    n_fix = 0
    for f in nc.m.functions:
        for blk in f.blocks:
            out = []
            for inst in blk.instructions:
                si = inst.sync_info
                if si and len(si.on_wait) > 1 and not isinstance(inst, mybir.InstNoOp):
                    waits = list(si.on_wait)
                    for w in waits[:-1]:
                        nop = mybir.InstNoOp(name=f"{inst.name}-ws{n_fix}", ins=[], outs=[])
                        nop.engine = inst.engine
                        nop.sync_info = mybir.SyncInfo(on_wait=[w], on_update=[])
                        out.append(nop)
                        n_fix += 1
                    si.on_wait = [waits[-1]]
                out.append(inst)
            blk.instructions[:] = out
    return n_fix


def host_prep(inputs):
    """Sort/shard edges, build per-core arrays and weight layouts."""
    Z = np.asarray(inputs["Z"]).astype(np.int32)
    edge_type = np.asarray(inputs["edge_type"]).astype(np.int32)
    dist = np.asarray(inputs["dist"]).astype(np.float32)
    src = np.asarray(inputs["src"]).astype(np.int64)
    dst = np.asarray(inputs["dst"]).astype(np.int64)
    graph_ids = np.asarray(inputs["graph_ids"]).astype(np.int64)

    order = np.argsort(dst, kind="stable")
    dsts = dst[order]
    srcs = src[order]
    dists = dist[order]
    etypes = edge_type[order]

    core_lo = np.searchsorted(dsts, np.arange(NCORES) * N_SH, side="left")
    core_hi = np.append(core_lo[1:], E)

    tile_cnt = np.zeros((NCORES, NT), dtype=np.int64)
    for c in range(NCORES):
        dl = dsts[core_lo[c]:core_hi[c]] - c * N_SH
        tile_cnt[c] = np.bincount(dl // P, minlength=NT)
    tmax = max(1, int(np.max((tile_cnt + P - 1) // P)))

    n_sub = NT * tmax                      # 128-edge sub-tiles per core
    n512 = (n_sub + 3) // 4                # 512-edge z-tiles per core
    n_sub_pad = n512 * 4
    e_slots = n_sub_pad * P

    # global padded row index into the allgathered hn table
    src_row = ((srcs // N_SH) * N_PAD + (srcs % N_SH)).astype(np.int32)

    def to_pf(arr):
        # [e_slots] -> [128, n_sub_pad]; element (p, s) = arr[s*128 + p]
        return np.ascontiguousarray(arr.reshape(n_sub_pad, P).T)

    core_in = []
    for c in range(NCORES):
        lo, hi = core_lo[c], core_hi[c]
        dl = (dsts[lo:hi] - c * N_SH).astype(np.int64)
        sr = np.zeros(e_slots, dtype=np.int32)
        doff = np.full(e_slots, PAD_OFF, dtype=np.float32)
        dd = np.zeros(e_slots, dtype=np.float32)
        et = np.zeros(e_slots, dtype=np.int32)
        start = 0
        for t in range(NT):
            cnt = int(tile_cnt[c, t])
            base = t * tmax * P
            sl = slice(start, start + cnt)
            sr[base:base + cnt] = src_row[lo:hi][sl]
            doff[base:base + cnt] = (dl[sl] - t * P).astype(np.float32)
            dd[base:base + cnt] = dists[lo:hi][sl]
            et[base:base + cnt] = etypes[lo:hi][sl]
            start += cnt
        assert start == hi - lo
        x3 = np.stack([dd, dd * dd, np.ones_like(dd)], 0).astype(np.float32)
        Zc = np.zeros(N_PAD, dtype=np.int32)
        Zc[:N_SH] = Z[c * N_SH:(c + 1) * N_SH]
        core_in.append(dict(
            src_row=to_pf(sr), dst_off=to_pf(doff), x3=x3, etype=to_pf(et),
            z_idx=np.ascontiguousarray(Zc.reshape(NT, P).T),
        ))

    w = {}
    centers = np.linspace(CUT_LO, CUT_HI, NC, dtype=np.float32)
    gap = np.float32(centers[1] - centers[0])
    w["A"] = np.stack([2.0 * centers / gap,
                       -np.ones(NC, np.float32) / gap,
                       -(centers ** 2) / gap], 0).astype(np.float32)   # [3, NC]
    w["node_emb"] = np.asarray(inputs["node_emb"]).astype(np.float32)
    w["edge_emb"] = np.asarray(inputs["edge_emb"]).astype(np.float32)
    for i in range(N_CONV):
        w[f"wn1t_{i}"] = np.ascontiguousarray(np.asarray(inputs["Wn1"][i]).T.astype(np.float32))
        w[f"wn2t_{i}"] = np.ascontiguousarray(np.asarray(inputs["Wn2"][i]).T.astype(np.float32))
        w[f"we1t_{i}"] = np.ascontiguousarray(np.asarray(inputs["We1"][i]).T.astype(np.float32))
        w[f"we2t_{i}"] = np.ascontiguousarray(np.asarray(inputs["We2"][i]).T.astype(np.float32))
        w[f"wct_{i}"] = np.ascontiguousarray(np.asarray(inputs["Wc"][i]).T.astype(np.float32))
        w[f"bn1_{i}"] = np.asarray(inputs["bn1"][i]).reshape(D, 1).astype(np.float32)
        w[f"bn2_{i}"] = np.asarray(inputs["bn2"][i]).reshape(D, 1).astype(np.float32)
        w[f"be1_{i}"] = np.asarray(inputs["be1"][i]).reshape(DE, 1).astype(np.float32)
        w[f"be2_{i}"] = np.asarray(inputs["be2"][i]).reshape(D, 1).astype(np.float32)
        w[f"bc_{i}"] = np.asarray(inputs["bc"][i]).reshape(1, D).astype(np.float32)
    w["wr1t"] = np.ascontiguousarray(np.asarray(inputs["Wr1"]).T.astype(np.float32))
    w["wr2t"] = np.ascontiguousarray(np.asarray(inputs["Wr2"]).T.astype(np.float32))
    w["br1"] = np.asarray(inputs["br1"]).reshape(D, 1).astype(np.float32)
    w["br2"] = np.full((D, 1), np.asarray(inputs["br2"]).reshape(()),
                       dtype=np.float32)

    meta = dict(tmax=tmax, n_sub=n_sub, n512=n512, n_sub_pad=n_sub_pad,
                e_slots=e_slots)
    return core_in, w, meta, graph_ids


def build_nc(meta):
    tmax, n512, n_sub = meta["tmax"], meta["n512"], meta["n_sub"]
    n_sub_pad, e_slots = meta["n_sub_pad"], meta["e_slots"]

    nc = bass.Bass(num_devices=NCORES)

    t_in = {}

    def inp(name, shp, dt=F32):
        t_in[name] = nc.dram_tensor(name, shp, dt, kind="ExternalInput")
        return t_in[name]

    src_row = inp("src_row", [P, n_sub_pad], I32)
    dst_off = inp("dst_off", [P, n_sub_pad], F32)
    x3 = inp("x3", [3, e_slots], F32)
    etype = inp("etype", [P, n_sub_pad], I32)
    z_idx = inp("z_idx", [P, NT], I32)
    A_t = inp("A", [3, NC], F32)
    node_emb = inp("node_emb", [20, D], F32)
    edge_emb = inp("edge_emb", [400, D], F32)
    for i in range(N_CONV):
        for nm, shp in (("wn1t", [D, D]), ("wn2t", [D, D]), ("we1t", [DE, DE]),
                        ("we2t", [DE, D]), ("wct", [D, D]), ("bn1", [D, 1]),
                        ("bn2", [D, 1]), ("be1", [DE, 1]), ("be2", [D, 1]),
                        ("bc", [1, D])):
            inp(f"{nm}_{i}", shp)
    inp("wr1t", [D, D]); inp("wr2t", [D, 1]); inp("br1", [D, 1]); inp("br2", [D, 1])
    r_out = nc.dram_tensor("r_out", [N_PAD, 1], F32, kind="ExternalOutput")

    e_fm = nc.dram_tensor("e_fm", [P, e_slots], F32, kind="Internal")
    ee_dram = [nc.dram_tensor(f"ee_{i}", [P, e_slots], F32, kind="Internal")
               for i in range(N_CONV)]
    cc_in = [nc.dram_tensor(f"cc_in_{i}", [N_PAD, D], F32, kind="Internal")
             for i in range(N_CONV)]
    cc_out = [nc.dram_tensor(f"cc_out_{i}", [NCORES * N_PAD, D], F32,
                             kind="Internal", addr_space="Shared")
              for i in range(N_CONV)]

    with tile.TileContext(nc) as tc:
        with (
            tc.tile_pool(name="const", bufs=1) as cp,
            tc.tile_pool(name="sb", bufs=3) as sb,
            tc.tile_pool(name="gat", bufs=6) as gp,
            tc.tile_pool(name="zr", bufs=2) as zp,
            tc.tile_pool(name="pv", bufs=1, space="PSUM") as pv,
            tc.tile_pool(name="pz1", bufs=2, space="PSUM") as pz1,
            tc.tile_pool(name="pz2", bufs=1, space="PSUM") as pz2,
            tc.tile_pool(name="ptp", bufs=1, space="PSUM") as ptp,
            tc.tile_pool(name="pm", bufs=2, space="PSUM") as pm,
            tc.tile_pool(name="pd", bufs=1, space="PSUM") as pd,
        ):
            # ---------------- constants in SBUF ----------------
            ident = cp.tile([P, P], F32)
            make_identity(nc, ident[:])
            iota_i = cp.tile([P, P], I32)
            nc.gpsimd.iota(iota_i[:], pattern=[[1, P]], base=0, channel_multiplier=0)
            iota_f = cp.tile([P, P], F32)
            nc.vector.tensor_copy(out=iota_f[:], in_=iota_i[:])
            ones_row = cp.tile([1, P], F32)
            nc.vector.memset(ones_row[:], 1.0)

            def load_const(name, shp):
                tl = cp.tile(shp, F32, tag=name)
                nc.sync.dma_start(out=tl[:], in_=t_in[name][:, :])
                return tl

            A_sb = load_const("A", [3, NC])
            wr1t_sb = load_const("wr1t", [D, D])
            wr2t_sb = load_const("wr2t", [D, 1])
            br1_sb = load_const("br1", [D, 1])
            br2_sb = load_const("br2", [D, 1])
            W = {}
            for i in range(N_CONV):
                for nm in ("wn1t", "wn2t", "wct", "bn1", "bn2", "be2", "bc"):
                    shp = {"wn1t": [D, D], "wn2t": [D, D], "wct": [D, D],
                           "bn1": [D, 1], "bn2": [D, 1], "be2": [D, 1],
                           "bc": [1, D]}[nm]
                    W[f"{nm}_{i}"] = load_const(f"{nm}_{i}", shp)
                # we1t [428,428] -> K-chunk tiles [<=128, 428]
                for k, (k0, k1) in enumerate(KCH):
                    tl = cp.tile([k1 - k0, DE], F32, tag=f"we1t_{i}_{k}")
                    nc.sync.dma_start(out=tl[:], in_=t_in[f"we1t_{i}"][k0:k1, :])
                    W[f"we1t_{i}_{k}"] = tl
                    tl2 = cp.tile([k1 - k0, D], F32, tag=f"we2t_{i}_{k}")
                    nc.sync.dma_start(out=tl2[:], in_=t_in[f"we2t_{i}"][k0:k1, :])
                    W[f"we2t_{i}_{k}"] = tl2
                be1 = cp.tile([P, 4], F32, tag=f"be1_{i}")  # 4 chunk-columns
                for k, (k0, k1) in enumerate(KCH):
                    nc.sync.dma_start(out=be1[:k1 - k0, k:k + 1],
                                      in_=t_in[f"be1_{i}"][k0:k1, :])
                W[f"be1_{i}"] = be1

            # persistent h in SBUF, feature-major [128, N_PAD]
            h_fm = cp.tile([P, N_PAD], F32, tag="h_fm")

            # ---------------- helpers ----------------
            def gather_transpose_to(dst_tile, dst_col, table, idx_col):
                """Gather 128 rows of `table` by idx_col [128,1] -> transpose ->
                write into dst_tile[:, dst_col:dst_col+128] (feature-major)."""
                g = gp.tile([P, D], F32, tag="gath")
                nc.gpsimd.indirect_dma_start(
                    out=g[:], out_offset=None, in_=table[:, :],
                    in_offset=bass.IndirectOffsetOnAxis(ap=idx_col, axis=0))
                pt = ptp.tile([P, P], F32, space="PSUM", tag="tp")
                nc.tensor.transpose(out=pt[:], in_=g[:], identity=ident[:])
                nc.scalar.copy(out=dst_tile[:, dst_col:dst_col + P], in_=pt[:])

            # ---------------- prologue: h0 init ----------------
            zi = cp.tile([P, NT], I32, tag="z_idx_sb")
            nc.sync.dma_start(out=zi[:], in_=z_idx[:, :])
            for t in range(NT):
                gather_transpose_to(h_fm, t * P, node_emb, zi[:, t:t + 1])

            # ---------------- prologue: e_fm build ----------------
            eti = cp.tile([P, n_sub_pad], I32, tag="etype_sb")
            nc.sync.dma_start(out=eti[:], in_=etype[:, :])
            for j in range(n512):
                ef = sb.tile([P, 4 * P], F32, tag="ef_build")
                for a in range(4):
                    s = j * 4 + a
                    gather_transpose_to(ef, a * P, edge_emb, eti[:, s:s + 1])
                nc.sync.dma_start(out=e_fm[:, j * 512:(j + 1) * 512], in_=ef[:])

            # dst_off + src_row resident in SBUF (used each layer)
            doff_sb = cp.tile([P, n_sub_pad], F32, tag="doff_sb")
            nc.sync.dma_start(out=doff_sb[:], in_=dst_off[:, :])
            sri = cp.tile([P, n_sub_pad], I32, tag="sri_sb")
            nc.sync.dma_start(out=sri[:], in_=src_row[:, :])

            # ---------------- layers ----------------
            for i in range(N_CONV):
                # --- node MLP: hn = relu(Wn1@h + bn1); Wn2@ + bn2 ---
                for j0 in range(0, N_PAD, 512):
                    wdt = min(512, N_PAD - j0)
                    ps1 = pz1.tile([P, 512], F32, space="PSUM", tag="pz1")
                    nc.tensor.matmul(out=ps1[:, :wdt], lhsT=W[f"wn1t_{i}"][:],
                                     rhs=h_fm[:, j0:j0 + wdt], start=True, stop=True)
                    zb = sb.tile([P, 512], F32, tag="nmlp_z")
                    nc.scalar.activation(out=zb[:, :wdt], in_=ps1[:, :wdt],
                                         func=AF.Relu, bias=W[f"bn1_{i}"][:, :1])
                    ps2 = pz2.tile([P, 512], F32, space="PSUM", tag="pz2")
                    nc.tensor.matmul(out=ps2[:, :wdt], lhsT=W[f"wn2t_{i}"][:],
                                     rhs=zb[:, :wdt], start=True, stop=True)
                    hnb = sb.tile([P, 512], F32, tag="nmlp_hn")
                    nc.scalar.activation(out=hnb[:, :wdt], in_=ps2[:, :wdt],
                                         func=AF.Identity, bias=W[f"bn2_{i}"][:, :1])
                    # transpose to node-major and ship to cc_in
                    for a in range(wdt // P):
                        pt = ptp.tile([P, P], F32, space="PSUM", tag="tp")
                        nc.tensor.transpose(out=pt[:], in_=hnb[:, a * P:(a + 1) * P],
                                            identity=ident[:])
                        hnm = sb.tile([P, P], F32, tag="hn_nm")
                        nc.vector.tensor_copy(out=hnm[:], in_=pt[:])
                        nc.sync.dma_start(
                            out=cc_in[i][j0 + a * P:j0 + (a + 1) * P, :], in_=hnm[:])

                nc.gpsimd.collective_compute(
                    "AllGather", ALU.bypass,
                    replica_groups=[list(range(NCORES))],
                    ins=[cc_in[i][:, :]], outs=[cc_out[i][:, :]])

                # --- ee z-chain (no dependence on h / collective) ---
                for j in range(n512):
                    js = slice(j * 512, (j + 1) * 512)
                    x3t = sb.tile([3, 512], F32, tag="x3t")
                    nc.sync.dma_start(out=x3t[:], in_=x3[:, js])
                    eft = sb.tile([P, 512], F32, tag="eft")
                    nc.sync.dma_start(out=eft[:], in_=e_fm[:, js])
                    # V chunks = exp(A.T @ x3)
                    vch = []
                    for k, (c0, c1) in enumerate(VCH):
                        pvt = pv.tile([P, 512], F32, space="PSUM", tag="pv")
                        nc.tensor.matmul(out=pvt[:c1 - c0, :], lhsT=A_sb[:, c0:c1],
                                         rhs=x3t[:], start=True, stop=True)
                        vt = sb.tile([P, 512], F32, tag=f"vch{k}")
                        nc.scalar.activation(out=vt[:c1 - c0, :], in_=pvt[:c1 - c0, :],
                                             func=AF.Exp)
                        vch.append(vt)
                    # z1 M-chunks, K = emb(128) + V(300)
                    z1r = []
                    for mi, (m0, m1) in enumerate(KCH):
                        pz = pz1.tile([P, 512], F32, space="PSUM", tag="pz1")
                        nc.tensor.matmul(out=pz[:m1 - m0, :],
                                         lhsT=W[f"we1t_{i}_0"][:, m0:m1],
                                         rhs=eft[:], start=True, stop=False)
                        for k, (c0, c1) in enumerate(VCH):
                            nc.tensor.matmul(
                                out=pz[:m1 - m0, :],
                                lhsT=W[f"we1t_{i}_{k + 1}"][:c1 - c0, m0:m1],
                                rhs=vch[k][:c1 - c0, :],
                                start=False, stop=(k == len(VCH) - 1))
                        zr_t = zp.tile([P, 512], F32, tag=f"z1r{mi}")
                        eng = nc.scalar if mi < 2 else nc.vector
                        if mi < 2:
                            nc.scalar.activation(out=zr_t[:m1 - m0, :], in_=pz[:m1 - m0, :],
                                                 func=AF.Relu,
                                                 bias=W[f"be1_{i}"][:m1 - m0, mi:mi + 1])
                        else:
                            nc.vector.tensor_scalar(
                                out=zr_t[:m1 - m0, :], in0=pz[:m1 - m0, :],
                                scalar1=W[f"be1_{i}"][:m1 - m0, mi:mi + 1],
                                scalar2=0.0, op0=ALU.add, op1=ALU.max)
                        z1r.append(zr_t)
                    # z2 = We2 @ z1r + be2 -> ee
                    pe = pz2.tile([P, 512], F32, space="PSUM", tag="pz2")
                    for k, (k0, k1) in enumerate(KCH):
                        nc.tensor.matmul(out=pe[:], lhsT=W[f"we2t_{i}_{k}"][:],
                                         rhs=z1r[k][:k1 - k0, :],
                                         start=(k == 0), stop=(k == len(KCH) - 1))
                    eet = sb.tile([P, 512], F32, tag="eet")
                    nc.scalar.activation(out=eet[:], in_=pe[:], func=AF.Identity,
                                         bias=W[f"be2_{i}"][:, :1])
                    nc.sync.dma_start(out=ee_dram[i][:, js], in_=eet[:])

                # --- consume: gather hn, product, message, scatter ---
                cur_pd = [None]
                for j in range(n512):
                    js = slice(j * 512, (j + 1) * 512)
                    eet = sb.tile([P, 512], F32, tag="eet_c")
                    nc.sync.dma_start(out=eet[:], in_=ee_dram[i][:, js])
                    hnf = sb.tile([P, 512], F32, tag="hnf")
                    n_active = min(4, n_sub - j * 4)
                    for a in range(n_active):
                        s = j * 4 + a
                        gather_transpose_to(hnf, a * P, cc_out[i], sri[:, s:s + 1])
                    prod = sb.tile([P, 512], F32, tag="prod")
                    nc.vector.tensor_mul(out=prod[:, :n_active * P], in0=eet[:, :n_active * P],
                                         in1=hnf[:, :n_active * P])
                    for a in range(n_active):
                        s = j * 4 + a
                        t_node = s // tmax
                        pos = s % tmax
                        if pos == 0:
                            cur_pd[0] = pd.tile([P, P], F32, space="PSUM", tag="pd", name="pdt")
                        pmt = pm.tile([P, P], F32, space="PSUM", tag="pm")
                        nc.tensor.matmul(out=pmt[:], lhsT=prod[:, a * P:(a + 1) * P],
                                         rhs=W[f"wct_{i}"][:], start=True, stop=False)
                        nc.tensor.matmul(out=pmt[:], lhsT=ones_row[:],
                                         rhs=W[f"bc_{i}"][:], start=False, stop=True)
                        msb = sb.tile([P, P], F32, tag="msb")
                        nc.scalar.activation(out=msb[:], in_=pmt[:], func=AF.Tanh)
                        S = sb.tile([P, P], F32, tag="S")
                        nc.vector.tensor_tensor(
                            out=S[:], in0=doff_sb[:, s:s + 1].to_broadcast([P, P]),
                            in1=iota_f[:], op=ALU.is_equal)
                        pdt = cur_pd[0]
                        nc.tensor.matmul(out=pdt[:], lhsT=S[:], rhs=msb[:],
                                         start=(pos == 0), stop=(pos == tmax - 1))
                        if pos == tmax - 1:
                            dsb = sb.tile([P, P], F32, tag="dsb")
                            nc.vector.tensor_copy(out=dsb[:], in_=pdt[:])
                            pt = ptp.tile([P, P], F32, space="PSUM", tag="tp")
                            nc.tensor.transpose(out=pt[:], in_=dsb[:], identity=ident[:])
                            nc.vector.tensor_add(
                                out=h_fm[:, t_node * P:(t_node + 1) * P],
                                in0=h_fm[:, t_node * P:(t_node + 1) * P], in1=pt[:])

            # ---------------- readout ----------------
            for j0 in range(0, N_PAD, 512):
                wdt = min(512, N_PAD - j0)
                ps1 = pz1.tile([P, 512], F32, space="PSUM", tag="pz1")
                nc.tensor.matmul(out=ps1[:, :wdt], lhsT=wr1t_sb[:],
                                 rhs=h_fm[:, j0:j0 + wdt], start=True, stop=True)
                qb = sb.tile([P, 512], F32, tag="qb")
                nc.scalar.activation(out=qb[:, :wdt], in_=ps1[:, :wdt],
                                     func=AF.Relu, bias=br1_sb[:, :1])
                for a in range(wdt // P):
                    prt = pm.tile([P, P], F32, space="PSUM", tag="pm")
                    nc.tensor.matmul(out=prt[:, :1], lhsT=qb[:, a * P:(a + 1) * P],
                                     rhs=wr2t_sb[:], start=True, stop=True)
                    rsb = sb.tile([P, 1], F32, tag="rsb")
                    nc.scalar.activation(out=rsb[:], in_=prt[:, :1], func=AF.Identity,
                                         bias=br2_sb[:, :1])
                    nc.sync.dma_start(out=r_out[j0 + a * P:j0 + (a + 1) * P, :],
                                      in_=rsb[:])
    return nc


_CACHE = {}


def _get_runner(meta):
    key = tuple(sorted(meta.items()))
    if key not in _CACHE:
        nc = build_nc(meta)
        nc.finalize()
        split_waits(nc)
        _CACHE[key] = nc
    return _CACHE[key]


def kernel(**inputs):
    core_in, w, meta, graph_ids = host_prep(inputs)
    nc = _get_runner(meta)
    in_maps = []
    for c in range(NCORES):
        m = dict(core_in[c])
        m.update(w)
        in_maps.append(m)
    res = run_bass_kernel_spmd(nc, in_maps, core_ids=list(range(NCORES)))
    r = np.concatenate([res.results[c]["r_out"][:N_SH, 0] for c in range(NCORES)])
    out = np.bincount(graph_ids, weights=r.astype(np.float64), minlength=G)[:G]
    return out.astype(np.float32)
